# revision 1
# baseline (speedup 1.0000x reference)
"""Trainium2 Bass kernel for: out = segment_sum(sigmoid(x @ w), segment_ids).

Shapes (hardcoded): x [1048576, 64] f32, w [64, 128] f32,
segment_ids [1048576] int32 (sorted), num_segments = 4096. Output [4096, 128] f32.

Strategy (8 cores, data parallel by bags):
  - 4096 bags -> 512 bags/core -> 16 windows of 32 bags per core.
  - Each window's items (avg 8192) are padded to NBW blocks of 128 items.
  - Host pre-layout: x is cast to bf16 and laid out block-transposed
    ([64 feat, 128 items] per block, two blocks stacked per 128 partitions)
    so the device never transposes. segment ids are rebased per window
    (value in [0,32) or -1 for padding).
  - Device per block: mm1 s_z = xT.T @ w -> PSUM f32, ACT sigmoid -> SBUF bf16,
    DVE builds onehot [item, bag_in_window] mask via is_equal, mm2 accumulates
    onehot.T @ s into PSUM [32, 128] over the whole window. One PSUM->SBUF
    copy + DMA per window. No collectives; host concatenates per-core outputs.
"""

import os

import numpy as np
import ml_dtypes

# problem constants (hardcoded per harness contract)
N = 1048576
F = 64
C = 128
B = 4096
NC = 8           # cores
BPC = B // NC    # bags per core = 512
W = 32           # bags per window
NW = BPC // W    # windows per core = 16
BLK = 128        # items per block
G = 12           # blocks per sigmoid/onehot group (3 PSUM banks)

bf16 = ml_dtypes.bfloat16


def _g_list(nbw):
    """Split nbw blocks into groups of 12 or 8 (each group = 3 or 2 PSUM
    banks; pairing block p with p+gn/2 keeps concurrent row-group matmuls
    in different banks). Returns None if nbw is not expressible."""
    n12 = nbw // 12
    while n12 >= 0:
        rem = nbw - 12 * n12
        if rem % 8 == 0:
            return [12] * n12 + [8] * (rem // 8)
        n12 -= 1
    return None


def _round_nbw(nbw):
    if nbw % 2:
        nbw += 1
    while _g_list(nbw) is None:
        nbw += 2
    return nbw


def _host_prepare(x, w, segment_ids):
    """Shard + relayout inputs for the 8 cores. Returns per-core input maps
    and the compile-time constant NBW (blocks per window)."""
    counts = np.bincount(segment_ids, minlength=B)
    off = np.zeros(B + 1, np.int64)
    off[1:] = np.cumsum(counts)

    n_items = off[W:][::W][: NC * NW * 1]  # noqa - computed below per window
    starts = off[:-1:W][: NC * NW]         # start offset of each 32-bag window
    ends = off[W::W][: NC * NW]
    per_win = (ends - starts).astype(np.int64)
    NBW = _round_nbw(int(-(-per_win.max() // BLK)))
    g_sizes = _g_list(NBW)
    NP2 = NBW // 2

    x_bf = x.astype(bf16)
    w_bf = w.astype(bf16)

    in_maps = []
    for k in range(NC):
        X = np.zeros((NW, 128, NP2 * BLK), bf16)
        SEG = np.full((128, NW * NBW), -1.0, np.float32)
        for wi in range(NW):
            widx = k * NW + wi
            i0, i1 = int(starts[widx]), int(ends[widx])
            n = i1 - i0
            xb = np.zeros((NBW * BLK, F), bf16)
            xb[:n] = x_bf[i0:i1]
            # [NBW,128,64] -> [NBW,64,128]; pair block p with p+gn/2 of its
            # group on partitions 0-63 / 64-127 (different PSUM banks)
            xb3 = xb.reshape(NBW, BLK, F).transpose(0, 2, 1)
            cols = []
            blk0 = 0
            for gn in g_sizes:
                half = gn // 2
                for p in range(half):
                    cols.append(np.concatenate(
                        [xb3[blk0 + p], xb3[blk0 + p + half]], axis=0))
                blk0 += gn
            X[wi] = np.concatenate(cols, axis=1)

            sa = np.full((NBW * BLK,), -1.0, np.float32)
            sa[:n] = (segment_ids[i0:i1] - (widx * W)).astype(np.float32)
            SEG[:, wi * NBW:(wi + 1) * NBW] = sa.reshape(NBW, BLK).T
        in_maps.append({
            "x_stream": X,
            "seg": SEG,
            "iota": np.tile(np.arange(W, dtype=np.float32), (128, 1)),
            "w_rep": np.concatenate([w_bf, w_bf], axis=0),
        })
    return in_maps, NBW


def _build_bass(NBW):
    import concourse.bass as bass
    import concourse.bacc as bacc
    import concourse.tile as tile
    from concourse import mybir

    NP2 = NBW // 2
    # Bacc (not plain Bass): its finalize() runs generate_event_semaphores,
    # which splits multi-sem waits (TRN2 allows 1 wait per instruction).
    nc = bacc.Bacc("TRN2", target_bir_lowering=False, debug=False)
    X = nc.dram_tensor("x_stream", [NW, 128, NP2 * BLK], mybir.dt.bfloat16,
                       kind="ExternalInput")
    SEG = nc.dram_tensor("seg", [128, NW * NBW], mybir.dt.float32,
                         kind="ExternalInput")
    IOTA = nc.dram_tensor("iota", [128, W], mybir.dt.float32,
                          kind="ExternalInput")
    WREP = nc.dram_tensor("w_rep", [128, C], mybir.dt.bfloat16,
                          kind="ExternalInput")
    OUT = nc.dram_tensor("out", [NW, W, C], mybir.dt.float32,
                         kind="ExternalOutput")

    g_sizes = _g_list(NBW)

    with tile.TileContext(nc) as tc:
        from contextlib import ExitStack
        with ExitStack() as ctx:
            const_pool = ctx.enter_context(tc.tile_pool(name="const", bufs=1))
            x_pool = ctx.enter_context(tc.tile_pool(name="xw", bufs=4))
            s_sb_pool = ctx.enter_context(tc.tile_pool(name="s_sb", bufs=3))
            oh_pool = ctx.enter_context(tc.tile_pool(name="oh", bufs=3))
            out_sb_pool = ctx.enter_context(tc.tile_pool(name="out_sb", bufs=2))
            s_ps_pool = ctx.enter_context(
                tc.tile_pool(name="s_ps", bufs=2, space="PSUM"))
            out_ps_pool = ctx.enter_context(
                tc.tile_pool(name="out_ps", bufs=2, space="PSUM"))

            iota_sb = const_pool.tile([128, W], mybir.dt.float32)
            nc.gpsimd.dma_start(iota_sb[:], IOTA[:])
            wrep_sb = const_pool.tile([128, C], mybir.dt.bfloat16)
            nc.gpsimd.dma_start(wrep_sb[:], WREP[:])
            seg_sb = const_pool.tile([128, NW * NBW], mybir.dt.float32)
            nc.gpsimd.dma_start(seg_sb[:], SEG[:])

            from collections import deque
            pending = deque()

            for wi in range(NW):
                out_ps = out_ps_pool.tile([W, C], mybir.dt.float32)
                blk0 = 0
                for gi, gn in enumerate(g_sizes):
                    npair = gn // 2
                    c0 = (blk0 // 2) * BLK
                    xw = x_pool.tile([128, npair * BLK], mybir.dt.bfloat16,
                                     tag="xw")
                    nc.gpsimd.dma_start(xw[:], X[wi, :, c0:c0 + npair * BLK])

                    s_ps = s_ps_pool.tile([128, gn * BLK], mybir.dt.float32,
                                          tag="s_ps")
                    for p in range(npair):
                        nc.tensor.matmul(
                            s_ps[:, p * BLK:(p + 1) * BLK],
                            lhsT=xw[0:64, p * BLK:(p + 1) * BLK],
                            rhs=wrep_sb[0:64, :],
                            start=True, stop=True)
                        nc.tensor.matmul(
                            s_ps[:, (p + npair) * BLK:(p + npair + 1) * BLK],
                            lhsT=xw[64:128, p * BLK:(p + 1) * BLK],
                            rhs=wrep_sb[64:128, :],
                            start=True, stop=True)

                    s_sb = s_sb_pool.tile([128, gn * BLK], mybir.dt.bfloat16,
                                          tag="s_sb")
                    nc.scalar.activation(s_sb[:], s_ps[:],
                                         mybir.ActivationFunctionType.Sigmoid)

                    oh = oh_pool.tile([128, gn * W], mybir.dt.bfloat16, tag="oh")
                    seg_slice = seg_sb[:, wi * NBW + blk0: wi * NBW + blk0 + gn]
                    nc.vector.tensor_tensor(
                        out=oh[:].rearrange("p (g w) -> p g w", w=W),
                        in0=seg_slice.unsqueeze(2).to_broadcast([128, gn, W]),
                        in1=iota_sb[:].unsqueeze(1).to_broadcast([128, gn, W]),
                        op=mybir.AluOpType.is_equal)

                    def mm2_group(oh=oh, s_sb=s_sb, out_ps=out_ps, gn=gn,
                                  blk0=blk0, wi=wi):
                        for j in range(gn):
                            nc.tensor.matmul(
                                out_ps[:],
                                lhsT=oh[:, j * W:(j + 1) * W],
                                rhs=s_sb[:, j * BLK:(j + 1) * BLK],
                                start=(blk0 + j == 0),
                                stop=(blk0 + j == NBW - 1),
                                skip_group_check=True)
                    pending.append(mm2_group)
                    blk0 += gn

                    while len(pending) > 1:
                        pending.popleft()()

                def finish_window(out_ps=out_ps, wi=wi):
                    out_sb = out_sb_pool.tile([W, C], mybir.dt.float32,
                                              tag="out_sb")
                    nc.vector.tensor_copy(out_sb[:], out_ps[:])
                    nc.gpsimd.dma_start(OUT[wi], out_sb[:])
                pending.append(finish_window)

            while pending:
                pending.popleft()()

    nc.finalize()
    return nc


def kernel(x, w, segment_ids, num_segments):
    x = np.ascontiguousarray(np.asarray(x, dtype=np.float32))
    w = np.ascontiguousarray(np.asarray(w, dtype=np.float32))
    segment_ids = np.ascontiguousarray(np.asarray(segment_ids, dtype=np.int32))
    assert int(num_segments) == B
    assert x.shape == (N, F) and w.shape == (F, C)

    from concourse.bass_utils import run_bass_kernel_spmd

    in_maps, NBW = _host_prepare(x, w, segment_ids)
    nc = _build_bass(NBW)

    trace = os.environ.get("KERNEL_TRACE", "0") == "1"
    res = run_bass_kernel_spmd(nc, in_maps, core_ids=list(range(NC)),
                               trace=trace)
    if trace and res.exec_time_ns is not None:
        print(f"HW exec time: {res.exec_time_ns} ns")
        if res.instructions_and_trace is not None:
            print(f"trace: {res.instructions_and_trace[1]}")

    out = np.concatenate(
        [r["out"].reshape(BPC, C) for r in res.results], axis=0)
    return out.astype(np.float32)

